# revision 1
# baseline (speedup 1.0000x reference)
"""ColBERT MaxSim late-interaction scoring on 8 Trainium2 NeuronCores.

scores[q, d] = sum_m max_n ( (Q*q_mask)[q,m,:] . (D*d_mask)[d,n,:] )

Sharding: candidate-parallel — the 512 docs are dealt across the 8 cores
(64 docs each); Q is replicated. Each core computes its [32, 64] score
block on device; the host concatenates.

Device algorithm (per core), all FLOPs on Trainium:
  1. The masks are folded into *layout*, not arithmetic:
     - doc tokens with d_mask=0 are simply not shipped; each doc's valid
       tokens are gathered (a permutation, no math) and zero-padded to a
       per-unit width w. A padded zero column yields sim==0, exactly what
       the reference's masked-out columns contribute to the max.
     - query-token rows with q_mask=0 contribute exactly 0 to the final
       sum (their sim row is all zeros before the max), so they are
       dropped; the surviving qm rows are packed into B blocks of 128.
       q attribution happens via a 0/1 indicator matrix in step 4.
  2. PE: sim[qm_block, doc, tok] = qt_b^T @ dg  (fp32 matmuls into PSUM)
  3. DVE: maxv[qm, doc] = reduce_max over tok (exact; includes the
     padded zeros, matching the reference's masked zeros)
  4. PE: scores[q, doc] += indicator_b^T @ maxv_b  (accumulated in PSUM
     over the B blocks — this is the sum over m, with q_mask applied via
     the 0/1 indicator, which is exact since masked rows max to 0)
"""

import sys

sys.path.insert(0, "/opt/trn_rl_repo")

import math
from contextlib import ExitStack

import numpy as np

import concourse.bass as bass
import concourse.mybir as mybir
from concourse.tile import TileContext
from concourse.tile_rust import add_dep_helper
from concourse.vector_clock import ScopedClock, VectorClock

N_CORES = 8
H = 128          # embedding dim == contraction dim == SBUF partitions
Q_N, M_N = 32, 32
D_N, T_N = 512, 180
P_DOCS = D_N // N_CORES          # 64 docs per core
PSUM_UNIT_COLS = 1536            # 3 PSUM banks per sim unit
MM_MAX_N = 512                   # fp32 moving-operand limit

F32 = mybir.dt.float32
F32R = mybir.dt.float32r

_N_PROCS = 27
_patched = False


def _install_tile_patch():
    """walrus rejects >2 sem waits on one CTRL: split the Tile tail drain
    into one SP drain per outstanding proc (SP executes them in order)."""
    global _patched
    if _patched:
        return
    _patched = True

    def _split_drain_and_barrier(self, tick_clock, wait_clock):
        nc = self.nc
        g = tick_clock.global_clock
        for p in range(_N_PROCS):
            t = g[p]
            if t > 0:
                d = nc.sync.drain()
                wait_clock.add_sem_waits(
                    d.ins,
                    ScopedClock(
                        {
                            None: VectorClock(
                                [t if i == p else 0 for i in range(_N_PROCS)]
                            )
                        }
                    ),
                )
        nc.sync.drain()
        nc.all_engine_barrier()
        assert self.sems is not None
        popped = nc._tile_sem_poison_stack.pop()
        assert popped is self._sem_poison
        nc.clear_and_free_semaphores(list(self.sems.allocated().values()))
        nc.all_engine_barrier()

    TileContext._drain_and_barrier = _split_drain_and_barrier


def _host_prep(Q, D, q_mask, d_mask):
    """Pure layout work: compaction gathers, padding, sharding. No FLOPs."""
    Q = np.asarray(Q, dtype=np.float32)
    D = np.asarray(D, dtype=np.float32)
    q_mask = np.asarray(q_mask)
    d_mask = np.asarray(d_mask)

    # ---- query side: pack valid qm rows into B blocks of 128
    qm_flat = q_mask.reshape(-1) != 0
    idx = np.flatnonzero(qm_flat)                       # valid qm rows, in order
    V = len(idx)
    B = max(1, math.ceil(V / 128))
    Qf = Q.reshape(Q_N * M_N, H)
    qt = np.zeros((H, B * 128), np.float32)
    if V:
        qt[:, :V] = Qf[idx].T
    wseg = np.zeros((H, B * Q_N), np.float32)           # [j, b*32+q] indicator
    for j, r in enumerate(idx):
        wseg[j % 128, (j // 128) * Q_N + (r // M_N)] = 1.0

    # ---- doc side: sort docs by valid-token count (desc), deal round-robin
    cnt = (d_mask != 0).sum(axis=1)
    order = np.argsort(-cnt, kind="stable")             # global doc order
    # position p on a core holds global ranks 8p..8p+7; width at position p
    wpos = cnt[order[0::N_CORES]]                       # [64] max count at each position

    # greedy units: consecutive positions sharing one padded width w.
    # Each unit spans nb full PSUM banks; every bank holds dpb = 512//w docs
    # (the unit is padded with zero-docs up to nb*dpb, so matmul/reduce APs
    # stay uniform and no matmul crosses a bank boundary).
    units = []                                          # (start_pos, nd, w, nb, dpb)
    p = 0
    while p < P_DOCS:
        w = max(int(wpos[p]), 1)
        w = (w + 7) // 8 * 8                            # multiple of 8
        dpb = max(1, MM_MAX_N // w)                     # docs per bank
        nd = min(3 * dpb, P_DOCS - p)                   # up to 3 banks per unit
        nb = math.ceil(nd / dpb)
        units.append((p, nd, w, nb, dpb))
        p += nd
    # dg layout: unit u occupies nb*dpb doc slots of width w (incl. zero pads)
    offs = np.cumsum([0] + [nb * dpb * w for _, _, w, nb, dpb in units])
    total_cols = int(offs[-1])
    base_cols = np.cumsum([0] + [nb * dpb for _, _, _, nb, dpb in units])
    p_pad = int(base_cols[-1])                          # padded doc-slot count

    # ---- per-core compacted D shard [H, total_cols]
    dgs = []
    tok_idx = [np.flatnonzero(d_mask[d]) for d in range(D_N)]
    for c in range(N_CORES):
        dg = np.zeros((H, total_cols), np.float32)
        for u, (start, nd, w, nb, dpb) in enumerate(units):
            off = int(offs[u])
            for k in range(nd):
                doc = order[(start + k) * N_CORES + c]
                tk = tok_idx[doc]
                if len(tk):
                    dg[:, off + k * w : off + k * w + len(tk)] = D[doc][tk].T
        dgs.append(dg)

    return dict(
        qt=qt, wseg=wseg, dgs=dgs, units=units, offs=offs, base_cols=base_cols,
        total_cols=total_cols, p_pad=p_pad, B=B, order=order,
    )


def _build_program(B, units, offs, base_cols, total_cols, p_pad, routes=None,
                  repeats=1, loop_n=None, compute_passes=1):
    """One SPMD program; per-core data comes via in_maps.

    routes: optional list of 'dve' | 'act_gp' per (unit-major, block-minor)
    flat index. 'dve' reduces PSUM directly on the vector engine; 'act_gp'
    stages PSUM->SBUF on the scalar engine, pre-reduces with a gpsimd
    pairwise-max tree, and finishes with a small DVE reduce.
    """
    _install_tile_patch()
    nc = bass.Bass(trn_type="TRN2")
    qt_d = nc.dram_tensor("qt", [H, B * 128], F32R, kind="ExternalInput")
    wseg_d = nc.dram_tensor("wseg", [H, B * Q_N], F32, kind="ExternalInput")
    dg_d = nc.dram_tensor("dg", [H, total_cols], F32R, kind="ExternalInput")
    out_d = nc.dram_tensor("out", [Q_N, p_pad], F32, kind="ExternalOutput")

    nu = len(units)
    if routes is None:
        routes = ["dve"] * (nu * B)
    BANK = 512

    with TileContext(nc) as tc, ExitStack() as ctx:
        const = ctx.enter_context(tc.tile_pool(name="const", bufs=1))
        dpool = ctx.enter_context(tc.tile_pool(name="dg", bufs=1))
        stage = ctx.enter_context(tc.tile_pool(name="stage", bufs=3))
        tree = ctx.enter_context(tc.tile_pool(name="tree", bufs=2))
        mpool = ctx.enter_context(tc.tile_pool(name="maxv", bufs=2))
        opool = ctx.enter_context(tc.tile_pool(name="out", bufs=2))
        psum = ctx.enter_context(tc.tile_pool(name="ps", bufs=2, space="PSUM"))
        pso_pool = ctx.enter_context(tc.tile_pool(name="pso", bufs=1, space="PSUM"))

        qt_t = const.tile([H, B * 128], F32R, tag="qt")
        qdma = nc.gpsimd.dma_start(out=qt_t[:], in_=qt_d[:, :])
        wseg_t = const.tile([H, B * Q_N], F32, tag="wseg")
        wdma = nc.gpsimd.dma_start(out=wseg_t[:], in_=wseg_d[:, :])
        # absorb the input-DMA waits into PE nops: the fp32r matmul's
        # S3_LW encoding only has room for one sem wait
        for dma in (qdma, wdma):
            n = nc.tensor.nop(hint="absorb_dma_wait")
            add_dep_helper(n.ins, dma.ins, sync=True)

        def pe_guard(*insts):
            """Absorb cross-engine waits into PE nops so the S3_LW matmul
            encoding (room for ~1 sem wait) never overflows."""
            for inst in insts:
                if inst is None:
                    continue
                n = nc.tensor.nop(hint="pe_guard")
                add_dep_helper(n.ins, inst.ins, sync=True)

        ps_hist = []        # (last matmul writer, reader) per flat psum tile
        prev_out_copy = None
        prev_dg_readers = None   # last matmul touching each dg tile, prev rep
        loop_cm = tc.For_i(0, loop_n, 1) if loop_n else None
        if loop_cm:
            loop_cm.__enter__()
        for _rep in range(repeats):
            dg_tiles, dg_dmas = [], []
            for u, (start, nd, w, nb, dpb) in enumerate(units):
                cols = nb * dpb * w
                dt = dpool.tile([H, cols], F32R, tag=f"dg{u}")
                if prev_dg_readers is not None and prev_dg_readers[u] is not None:
                    # absorb the WAR wait (prev rep's matmul reader) into an
                    # SP nop: the DMA encoding has room for only one wait
                    sn = nc.sync.nop(hint="sp_guard")
                    add_dep_helper(sn.ins, prev_dg_readers[u].ins, sync=True)
                # sync (SP/HWDGE) keeps the Pool engine free for max-trees
                udma = nc.sync.dma_start(
                    out=dt[:], in_=dg_d[:, int(offs[u]) : int(offs[u]) + cols]
                )
                dg_tiles.append(dt)
                dg_dmas.append(udma)
            dg_readers = [None] * len(units)

            # compute_passes > 1 repeats compute on resident dg tiles (for
            # steady-state benchmarking without DMA WAR chains)
            for _pass in range(compute_passes):
                maxv = mpool.tile([H, B, p_pad], F32, tag="maxv")
                reduces = []
                for u, (start, nd, w, nb, dpb) in enumerate(units):
                    base = int(base_cols[u])
                    for b in range(B):
                        f = len(ps_hist)
                        nc.tensor.nop(hint="spare")
                        nc.vector.nop(hint="spare")
                        if b == 0:
                            nc.sync.nop(hint="spare")
                            nc.scalar.nop(hint="spare")
                        # guards: psum-slot WAR + prior writer completion
                        # (bufs=2 -> slot f-2), plus this unit's dg load
                        guards = []
                        if f >= 2:
                            guards.extend(ps_hist[f - 2])
                        if b == 0:
                            guards.append(dg_dmas[u])
                        pe_guard(*guards)
                        ps = psum.tile([H, nb * BANK], F32, tag="ps")
                        last_mm = None
                        for k in range(nb):                 # one matmul per bank
                            # float32r: 1 cycle/column on the PE (plain float32
                            # runs as two half-speed passes = 4 cyc/col)
                            last_mm = nc.tensor.matmul(
                                ps[:, k * BANK : k * BANK + dpb * w],
                                lhsT=qt_t[:, b * 128 : (b + 1) * 128],
                                rhs=dg_tiles[u][:, k * dpb * w : (k + 1) * dpb * w],
                                start=True,
                                stop=True,
                            )
                        dg_readers[u] = last_mm
                        # 4D view: [p, nb, dpb, w] with bank stride 512
                        ps4 = (
                            ps[:, :]
                            .rearrange("p (nb bank) -> p nb bank", bank=BANK)[
                                :, :, 0 : dpb * w
                            ]
                            .rearrange("p nb (d w) -> p nb d w", w=w)
                        )
                        mv_out = maxv[:, b, base : base + nb * dpb].rearrange(
                            "p (nb d) -> p nb d", d=dpb
                        )
                        if routes[u * B + b] == "dve":
                            rd = nc.vector.reduce_max(
                                out=mv_out, in_=ps4, axis=mybir.AxisListType.X
                            )
                            ps_hist.append((last_mm, rd))
                            reduces.append(rd)
                        else:  # act_gp: ACT stage -> gpsimd max tree -> DVE tail
                            st = stage.tile([H, nb * dpb * w], F32, tag="stage")
                            cp = nc.scalar.copy(
                                out=st[:].rearrange("p (nb dw) -> p nb dw", nb=nb),
                                in_=ps[:, :].rearrange("p (nb bank) -> p nb bank", bank=BANK)[
                                    :, :, 0 : dpb * w
                                ],
                            )
                            ps_hist.append((last_mm, cp))
                            cur = st[:].rearrange("p (d w) -> p d w", w=w)
                            wl = w
                            while wl > 8 and wl % 2 == 0:
                                half = wl // 2
                                nxt = tree.tile([H, nb * dpb, half], F32, tag=f"tr{half}")
                                nc.gpsimd.tensor_max(
                                    nxt[:, :, :], cur[:, :, :half], cur[:, :, half:wl]
                                )
                                cur = nxt[:, :, :]
                                wl = half
                            rd = nc.vector.reduce_max(
                                out=maxv[:, b, base : base + nb * dpb],
                                in_=cur,
                                axis=mybir.AxisListType.X,
                            )
                            reduces.append(rd)

                # guards for the segment matmuls: all maxv writers (dedups to
                # one DVE wait) + previous repeat's pso reader
                pe_guard(prev_out_copy)
                seg_guard = nc.tensor.nop(hint="seg_guard")
                for r in reduces:
                    add_dep_helper(seg_guard.ins, r.ins, sync=True)
                pso = pso_pool.tile([Q_N, p_pad], F32, tag="pso")
                for b in range(B):
                    nc.tensor.matmul(
                        pso[:, :],
                        lhsT=wseg_t[:, b * Q_N : (b + 1) * Q_N],
                        rhs=maxv[:, b, :],
                        start=(b == 0),
                        stop=(b == B - 1),
                    )
                if _pass == compute_passes - 1:
                    out_t = opool.tile([Q_N, p_pad], F32, tag="out")
                    oc = nc.scalar.copy(out=out_t[:], in_=pso[:, :])
                    prev_out_copy = oc
                    nc.gpsimd.nop(hint="spare")
                    nc.gpsimd.dma_start(out=out_d[:, :], in_=out_t[:])
            prev_dg_readers = dg_readers
        if loop_cm:
            loop_cm.__exit__(None, None, None)

    _redistribute_waits(nc)
    return nc


# walrus encoding limits on sem waits per instruction (observed: S3_LW
# matmul rejects 2, CTRL rejects 3)
_WAIT_CAPS = {"InstMatmult": 1, "InstNoOp": 1, "InstDrain": 1,
              "InstDMACopy": 1, "InstTensorReduce": 1, "InstActivation": 1,
              "InstTensorTensor": 1, "InstMemset": 1, "InstTensorCopy": 1}


def _redistribute_waits(nc):
    """Move excess sem waits off over-limit instructions onto earlier
    instructions of the same engine (in final program order). An earlier
    wait is strictly more conservative: the engine is in-order, so every
    dependency still holds; it can only cost stall time, and the producers
    of these sems never depend on the small window being skipped."""
    import dataclasses

    fn = nc.m.functions[0]
    streams = {}
    for bb in fn.blocks:
        for inst in bb.instructions:
            eng = inst.engine
            streams.setdefault(str(eng), []).append(inst)

    for eng, insts in streams.items():
        for i, inst in enumerate(insts):
            cap = _WAIT_CAPS.get(type(inst).__name__)
            si = inst.sync_info
            if cap is None or si is None or len(si.on_wait) <= cap:
                continue
            eng_name = str(inst.engine).split(".")[-1]
            # a wait on this engine's own completion sem can never move to
            # an earlier instruction of the same engine (it may await the
            # receiver itself -> cycle); keep self-waits in place first
            waits = sorted(
                si.on_wait,
                key=lambda w: 0 if w.ant_name.startswith(eng_name) else 1,
            )
            keep, excess = list(waits[:cap]), list(waits[cap:])
            if any(w.ant_name.startswith(eng_name) for w in excess):
                # more self-waits than capacity: cannot fix safely
                keep, excess = list(waits), []
            moved = []
            for w in excess:
                placed = False
                # window-limited backward move: crossing many instructions
                # risks a wait cycle with cross-repeat WAR chains
                for j in range(i - 1, max(-1, i - 7), -1):
                    p = insts[j]
                    # only move onto types whose encodings hold a sem wait
                    if type(p).__name__ not in (
                        "InstNoOp", "InstMatmult", "InstDrain",
                        "InstActivation", "InstTensorReduce", "InstTensorTensor",
                    ):
                        continue
                    pcap = _WAIT_CAPS.get(type(p).__name__, 1)
                    psi = p.sync_info
                    pw = list(psi.on_wait) if psi else []
                    merged = False
                    for k, ow in enumerate(pw):
                        if ow.id == w.id and ow.wait_mode == w.wait_mode == "sem-ge-imm":
                            pw[k] = dataclasses.replace(
                                ow, wait_value=max(ow.wait_value, w.wait_value)
                            )
                            merged = True
                            break
                    if not merged:
                        if len(pw) >= pcap:
                            continue
                        pw.append(w)
                    if psi is None:
                        psi = type(si)(on_wait=pw, on_update=[])
                    else:
                        psi = dataclasses.replace(psi, on_wait=pw)
                    p.sync_info = psi
                    placed = True
                    break
                if not placed:
                    keep.append(w)   # nowhere to put it; leave (will fail
                                     # compile loudly rather than silently)
                moved.append((w.ant_name, placed))
            inst.sync_info = dataclasses.replace(si, on_wait=keep)


def _run(nc, prep, n_cores=N_CORES):
    from concourse.bass_utils import run_bass_kernel_spmd

    in_maps = [
        {"qt": prep["qt"], "wseg": prep["wseg"], "dg": prep["dgs"][c]}
        for c in range(n_cores)
    ]
    res = run_bass_kernel_spmd(nc, in_maps, core_ids=list(range(n_cores)))
    return res.results


def _assemble(prep, results):
    order = prep["order"]
    base_cols = prep["base_cols"]
    scores = np.zeros((Q_N, D_N), np.float32)
    for c in range(N_CORES):
        out_c = results[c]["out"]                      # [32, p_pad]
        for u, (start, nd, w, nb, dpb) in enumerate(prep["units"]):
            base = int(base_cols[u])
            for k in range(nd):
                scores[:, order[(start + k) * N_CORES + c]] = out_c[:, base + k]
    return scores


def make_routes(B, units, use_act_gp=False):
    """Greedy engine-balance routing for the reduce stage."""
    nu = len(units)
    routes = ["dve"] * (nu * B)
    if not use_act_gp:
        return routes
    dve_busy, gp_busy = 0.0, 0.0
    for u, (start, nd, w, nb, dpb) in enumerate(units):
        npad = nb * dpb
        for b in range(B):
            fd = npad * w
            dve_direct = (120 + fd) / 0.96
            # act_gp: gpsimd tree (0.42 eff 2-input) + dve tail
            wl = w
            gp = 0.0
            while wl > 8 and wl % 2 == 0:
                wl //= 2
                gp += (npad * wl) / (0.42 * 1.2) + 300
            tail = (58 + npad * wl) / 0.96
            if max(dve_busy + dve_direct, gp_busy) <= max(
                dve_busy + tail, gp_busy + gp
            ):
                dve_busy += dve_direct
            else:
                gp_busy += gp
                dve_busy += tail
                routes[u * B + b] = "act_gp"
    return routes


_cache = {}


def kernel(Q, D, q_mask, d_mask):
    prep = _host_prep(Q, D, q_mask, d_mask)
    key = (prep["B"], tuple(prep["units"]))
    if key not in _cache:
        routes = make_routes(prep["B"], prep["units"])
        _cache[key] = _build_program(
            prep["B"], prep["units"], prep["offs"], prep["base_cols"],
            prep["total_cols"], prep["p_pad"], routes=routes,
        )
    nc = _cache[key]
    results = _run(nc, prep)
    return _assemble(prep, results)

